# revision 30
# baseline (speedup 1.0000x reference)
"""CBOW forward (mean-embed -> linear -> linear -> log_softmax) on 8 trn2 cores.

Vocab-parallel tensor parallelism: each core owns a V/8 = 4000-wide vocab shard.
Layer-1 partial h is AllReduced (64 KB); layer-2 + softmax statistics are
computed shard-locally with a tiny AllGather of per-core sum(exp(logits)).

v2 structure (vs the earlier selector-matmul version):
 - All big inputs are cast to bf16 AND pre-transposed on the host, halving
   ingest bytes (16.4 -> 8.2 MB/core) and removing all on-chip fp32->bf16
   cast traffic.  X arrives as X^T chunks [128v, 512row]; the context mean
   is a DVE grouped reduce (free-dim groups of 8), not a PE pass.  The 1/8
   mean scale is folded into W1 on the host.
 - Latency-critical small DMAs (biases, h bounce, sumexp bounces) ride the
   scalar HWDGE queue; bulk X/W1/W2 ingest rides the sync HWDGE queue, in
   the order layer-1 consumes it (W1 quarter before its X groups).
 - Logits are computed into [128, 500] PSUM tiles holding TWO 500-wide vocab
   chunks stacked on the partition axis (batch b = p%64), so exp runs at the
   full 128-partition ACT rate and b2 adds via a K=1 ones-matmul.
 - log(sumexp): per-core partial sums pack to [4, 32] via a DVE stream
   transpose, AllGather, local reduce, unpack to a [128, 1] per-partition
   scalar; the final subtract splits across DVE/ACT per chunk, each chunk's
   output DMA overlapping the next chunk's subtract.
 - No warmup collective: measured traces show the first ncfw collective is
   gated by global ingest drain, and a warmup mesh only serializes ahead of
   the real AllReduce.  Keep-warm matmuls hold the PE clock across the AR.

Problem shapes (hardcoded): B=64, 2N=8 context slots, V=32000, D=256, fp32 IO.
"""

import numpy as np

import concourse.bacc as bacc
import concourse.mybir as mybir
import concourse.tile as tile
from concourse.bass_utils import run_bass_kernel_spmd

N_CORES = 8
B = 64          # batch
NCTX = 8        # 2N context slots
V = 32000
D = 256
VS = V // N_CORES          # 4000 vocab columns per core
ROWS = B * NCTX            # 512 input rows, row = b*NCTX + i
NVC = 32                   # stage-1 v-chunks of 128 (4000 padded to 4096)
LC = 500                   # layer-2 logits chunk width; 8 chunks
N_WARM_MM = 55             # keep-warm matmuls covering the AllReduce gap
F32 = mybir.dt.float32
BF16 = mybir.dt.bfloat16

_cache = {}


def _build():
    nc = bacc.Bacc("TRN2", target_bir_lowering=False, debug=False,
                   num_devices=N_CORES)

    XP = nc.dram_tensor("xp", [128, NVC, ROWS], BF16, kind="ExternalInput")
    W1TP = nc.dram_tensor("w1tp", [128, NVC, D], BF16, kind="ExternalInput")
    W2TP = nc.dram_tensor("w2tp", [128, 2, VS], BF16, kind="ExternalInput")
    B2B = nc.dram_tensor("b2b", [128, 4 * LC], F32, kind="ExternalInput")
    B1T = nc.dram_tensor("b1t", [128, 2], F32, kind="ExternalInput")
    I64 = nc.dram_tensor("i64", [64, 64], BF16, kind="ExternalInput")
    OUT = nc.dram_tensor("out", [128, 4 * LC], F32, kind="ExternalOutput")

    rg = [list(range(N_CORES))]

    with tile.TileContext(nc) as tc:
        with (
            tc.tile_pool(name="consts", bufs=1) as consts,
            tc.tile_pool(name="xin", bufs=1) as xin,
            tc.tile_pool(name="wpool", bufs=1) as wpool,
            tc.tile_pool(name="work", bufs=1) as work,
            tc.tile_pool(name="dram", bufs=1, space="DRAM") as dram,
        ):
            # Small latency-critical loads at the head of the scalar ring.
            b1_sb = consts.tile([128, 2], F32)
            nc.scalar.dma_start(b1_sb[:], B1T.ap())
            i64_sb = consts.tile([64, 64], BF16)
            nc.scalar.dma_start(i64_sb[:], I64.ap())
            tbl_out = consts.tile([1, 2], F32)

            # Stage 1: xbar^T[v, b] = sum_i X^T[v, b*8+i] via DVE grouped
            # reduce; layer 1 h[b, d] += xbar^T[v, b]^T @ W1T[v, d] on PE.
            # Sync-queue order: W1 quarter q ahead of the X groups it feeds.
            # Ingest splits across BOTH HWDGE rings (sync + scalar), X groups
            # and W1 quarters interleaved so layer-1 inputs land earliest and
            # the rings fill each other's completion-receipt bubbles; W2 and
            # the bias tile ride the ring tails (needed only post-AllReduce).
            w1_sb = wpool.tile([128, NVC, D], BF16)
            w2_sb = wpool.tile([128, 2, VS], BF16)
            b2b_sb = wpool.tile([128, 4 * LC], F32)
            xbar_sb = work.tile([128, NVC * B], BF16)
            h_sb = work.tile([B, D], BF16)
            # pack-path scratch, zeroed early so the post-exp critical path
            # only pays the copy + transpose
            tr_in = work.tile([128, 32], F32)
            nc.vector.memset(tr_in[:], 0.0)
            xg = [xin.tile([128, 4, ROWS], BF16, name=f"xg{g}")
                  for g in range(8)]

            def w1q(q):
                return (w1_sb[:, 8 * q:8 * q + 8, :],
                        W1TP.ap()[:, 8 * q:8 * q + 8, :])

            def xgd(g):
                return (xg[g][:], XP.ap()[:, 4 * g:4 * g + 4, :])

            for eng, src in [
                (nc.sync, xgd(0)), (nc.scalar, w1q(0)),
                (nc.scalar, xgd(1)), (nc.sync, xgd(2)),
                (nc.scalar, w1q(1)), (nc.sync, xgd(4)),
                (nc.scalar, xgd(3)), (nc.sync, w1q(2)),
                (nc.scalar, xgd(5)), (nc.sync, xgd(6)),
                (nc.scalar, xgd(7)), (nc.sync, w1q(3)),
                (nc.sync, (w2_sb[:, 0, :], W2TP.ap()[:, 0, :])),
                (nc.scalar, (w2_sb[:, 1, :], W2TP.ap()[:, 1, :])),
                (nc.scalar, (b2b_sb[:], B2B.ap())),
            ]:
                eng.dma_start(src[0], src[1])

            with tc.tile_pool(name="ps1", bufs=1, space="PSUM") as ps1:
                h_ps = ps1.tile([B, D], F32)
                for g in range(8):
                    for j in range(4):
                        c = 4 * g + j
                        with nc.allow_low_precision(
                                reason="bf16 xbar; 8-term sums, matmul "
                                       "operands are bf16 anyway"):
                            nc.vector.reduce_sum(
                                xbar_sb[:, 64 * c:64 * c + 64],
                                xg[g][:, j, :].rearrange("p (b i) -> p b i",
                                                         i=NCTX),
                                axis=mybir.AxisListType.X)
                        nc.tensor.matmul(
                            h_ps[:],
                            xbar_sb[:, 64 * c:64 * c + 64],
                            w1_sb[:, c, :],
                            start=(c == 0), stop=(c == NVC - 1),
                        )
                with nc.allow_low_precision(
                        reason="bf16 h exchange halves the AllReduce mesh "
                               "payload; CCE adds 8 partials, ~0.4% rel"):
                    nc.vector.tensor_copy(h_sb[:], h_ps[:])

            # AllReduce partial h across the 8 vocab shards.
            hb_in = dram.tile([B, D], BF16)
            hb_out = dram.tile([B, D], BF16, addr_space="Shared")
            nc.sync.dma_start(hb_in[:], h_sb[:])
            nc.gpsimd.collective_compute(
                "AllReduce", mybir.AluOpType.add, replica_groups=rg,
                ins=[hb_in.opt()], outs=[hb_out.opt()])
            # hsum returns as two ring-parallel halves so each transpose can
            # start as soon as its own half lands.
            hsum_sb = work.tile([B, D], BF16)
            nc.sync.dma_start(hsum_sb[:, 0:128], hb_out[:, 0:128])
            nc.scalar.dma_start(hsum_sb[:, 128:256], hb_out[:, 128:256])

            # Keep-warm matmuls hold the PE activity monitor at full clock
            # across the AllReduce gap; then h^T via PE transpose, + b1 fused
            # into the PSUM->SBUF copy (cast to bf16 for layer 2).
            hT_sb = work.tile([128, 2, B], BF16)
            with tc.tile_pool(name="ps2", bufs=1, space="PSUM") as ps2:
                warm_ps = ps2.tile([B, 512], F32, tag="warm")
                for _ in range(N_WARM_MM):
                    nc.tensor.matmul(warm_ps[:], xbar_sb[:, 0:64],
                                     xbar_sb[:, 0:512], start=True, stop=True)
                for dc in range(2):
                    hT_ps = ps2.tile([128, B], BF16, tag="hT")
                    nc.tensor.transpose(
                        hT_ps[:], hsum_sb[:, dc * 128:(dc + 1) * 128],
                        i64_sb[:])
                    nc.vector.tensor_scalar_add(
                        hT_sb[:, dc, :], hT_ps[:], b1_sb[:, dc:dc + 1])

            # Layer 2 + log-softmax.  Logits land in 4 PSUM tiles [128, 500],
            # vocab chunks 2j (partitions 0:64) and 2j+1 (64:128), batch is
            # p % 64, so exp runs 128 partitions wide.  b2 adds on DVE (a
            # host-prebroadcast [128, 2000] tile) - off the PE chain.
            e_sb = work.tile([128, 4 * LC], BF16)
            lb_sb = work.tile([128, 4 * LC], F32)
            out_sb = work.tile([128, 4 * LC], F32)
            sums_sb = work.tile([128, 4], F32)

            with tc.tile_pool(name="ps3", bufs=1, space="PSUM") as ps3:
                # [128, 512] tiles (full PSUM bank) with 500 used columns:
                # a 2000 B tile would cross the 2 KB bank boundary and
                # silently corrupt matmul accumulation.
                lg = [ps3.tile([128, 512], F32, tag=f"lg{j}", name=f"lg{j}")
                      for j in range(4)]
                for j in range(4):
                    for half in range(2):
                        ch = 2 * j + half
                        ap = lg[j][64 * half:64 * half + 64, 0:LC]
                        for dc in range(2):
                            nc.tensor.matmul(
                                ap, hT_sb[:, dc, :],
                                w2_sb[:, dc, LC * ch:LC * ch + LC],
                                start=(dc == 0), stop=(dc == 1))
                    nc.vector.tensor_tensor(
                        lb_sb[:, LC * j:LC * j + LC], lg[j][:, 0:LC],
                        b2b_sb[:, LC * j:LC * j + LC],
                        op=mybir.AluOpType.add)
                    # logits are O(+-3) so fp32 exp needs no max-subtraction
                    nc.scalar.activation(
                        e_sb[:, LC * j:LC * j + LC],
                        lb_sb[:, LC * j:LC * j + LC],
                        mybir.ActivationFunctionType.Exp,
                        accum_out=sums_sb[:, j:j + 1])

                # Pull the Ln activation table in during the AllGather wait
                # (a function switch reloads the ACT table, ~1.3us).
                nc.scalar.activation(tbl_out[:, 0:1], sums_sb[0:1, 3:4],
                                     mybir.ActivationFunctionType.Ln)

                # Global sumexp: pack the [128, 1] per-partition partials
                # onto 4 partition rows (contiguous 512 B bounce), AllGather,
                # then one [K=8] ones-matmul sums the cores and restores the
                # [128, 1] per-partition layout in PSUM.
                s1 = work.tile([128, 1], F32)
                nc.vector.reduce_sum(s1[:], sums_sb[:],
                                     axis=mybir.AxisListType.X)
                nc.vector.tensor_copy(tr_in[:, 0:1], s1[:])
                tr_out = work.tile([128, 32], F32)
                nc.vector.transpose(tr_out[:], tr_in[:])
                sb_in = dram.tile([4, 32], F32)
                sb_out = dram.tile([N_CORES, 4, 32], F32, addr_space="Shared")
                nc.sync.dma_start(sb_in[:], tr_out[0:128:32, :])
                nc.gpsimd.collective_compute(
                    "AllGather", mybir.AluOpType.bypass, replica_groups=rg,
                    ins=[sb_in.opt()], outs=[sb_out.opt()])
                # Rows 0:8 = each core's packed s1[128]; rows 8:16 the same
                # with the two partition-halves swapped, so the ones-matmul
                # sums over cores AND halves: s_ps[p] = total sumexp of
                # batch p%64 on every partition.
                s16_sb = work.tile([2 * N_CORES, 128], F32)
                nc.sync.dma_start(s16_sb[0:8, :],
                                  sb_out[:].rearrange("r h b -> r (h b)"))
                nc.scalar.dma_start(
                    s16_sb[8:16, 0:64],
                    sb_out[:, 2:4, :].rearrange("r h b -> r (h b)"))
                nc.sync.dma_start(
                    s16_sb[8:16, 64:128],
                    sb_out[:, 0:2, :].rearrange("r h b -> r (h b)"))
                ones16_sb = consts.tile([2 * N_CORES, 1], F32)
                nc.vector.memset(ones16_sb[:], 1.0)
                s_ps = ps3.tile([128, 1], F32)
                nc.tensor.matmul(s_ps[:], s16_sb[:], ones16_sb[:],
                                 start=True, stop=True)
                ln_sb = work.tile([128, 1], F32)
                nc.scalar.activation(ln_sb[:], s_ps[:],
                                     mybir.ActivationFunctionType.Ln)
                negln_sb = work.tile([128, 1], F32)
                nc.vector.tensor_scalar_mul(negln_sb[:], ln_sb[:], -1.0)

                # out = logits - log(sumexp): chunks alternate DVE/ACT, each
                # chunk's output DMA overlaps the next chunk's subtract.
                for j in range(4):
                    dst = out_sb[:, LC * j:LC * j + LC]
                    src = lb_sb[:, LC * j:LC * j + LC]
                    if j % 2 == 0:
                        nc.vector.tensor_scalar_sub(dst, src,
                                                    ln_sb[:, 0:1])
                    else:
                        nc.scalar.activation(
                            dst, src,
                            mybir.ActivationFunctionType.Identity,
                            bias=negln_sb[:, 0:1])
                    dma_eng = nc.sync if j % 2 == 0 else nc.scalar
                    dma_eng.dma_start(OUT.ap()[:, LC * j:LC * j + LC], dst)

    nc.compile()
    return nc


def _get_nc():
    if "nc" not in _cache:
        _cache["nc"] = _build()
    return _cache["nc"]


def _make_in_maps(input_vec, W1, b1, W2, b2):
    import ml_dtypes

    input_vec = np.asarray(input_vec, dtype=np.float32)
    W1 = np.asarray(W1, dtype=np.float32)
    b1 = np.asarray(b1, dtype=np.float32)
    W2 = np.asarray(W2, dtype=np.float32)
    b2 = np.asarray(b2, dtype=np.float32)

    xr = input_vec.reshape(B, NCTX, V)
    i64 = np.eye(64, dtype=np.float32).astype(ml_dtypes.bfloat16)
    b1t = np.ascontiguousarray(b1.reshape(2, 128).T)

    in_maps = []
    for c in range(N_CORES):
        lo, hi = c * VS, (c + 1) * VS
        # X^T padded to 4096 rows, chunked [128, 32, 512]
        xt = xr[:, :, lo:hi].reshape(ROWS, VS).T        # [4000, 512]
        xtp = np.zeros((NVC * 128, ROWS), np.float32)
        xtp[:VS] = xt
        xp = np.ascontiguousarray(
            xtp.reshape(NVC, 128, ROWS).transpose(1, 0, 2)
        ).astype(ml_dtypes.bfloat16)
        # W1 shard, transposed + 1/8 context-mean folded in, padded like X
        w1s = (W1[:, lo:hi].T / NCTX).astype(np.float32)   # [4000, 256]
        w1p = np.zeros((NVC * 128, D), np.float32)
        w1p[:VS] = w1s
        w1tp = np.ascontiguousarray(
            w1p.reshape(NVC, 128, D).transpose(1, 0, 2)
        ).astype(ml_dtypes.bfloat16)
        w2tp = np.ascontiguousarray(
            W2[lo:hi, :].T.reshape(2, 128, VS).transpose(1, 0, 2)
        ).astype(ml_dtypes.bfloat16)
        # bias pre-broadcast matching the [128, 500]-pair logits layout:
        # b2b[64h+b, 500j+c] = b2[lo + 500*(2j+h)+c]
        seg = b2[lo:hi].reshape(8, LC)
        b2b = np.zeros((128, 4 * LC), np.float32)
        for h in range(2):
            for j in range(4):
                b2b[64 * h:64 * h + 64, LC * j:LC * j + LC] = seg[2 * j + h]
        in_maps.append({
            "xp": xp, "w1tp": w1tp, "w2tp": w2tp, "b2b": b2b,
            "b1t": b1t, "i64": i64,
        })
    return in_maps


def kernel(input_vec, W1, b1, W2, b2, **_unused):
    in_maps = _make_in_maps(input_vec, W1, b1, W2, b2)
    _cache["in_maps"] = in_maps
    nc = _get_nc()
    res = run_bass_kernel_spmd(nc, in_maps, core_ids=list(range(N_CORES)))
    # per-core result r[128, 2000]: out[b, 500*(2j+h)+c] = r[64h+b, 500j+c]
    outs = []
    for c in range(N_CORES):
        r = res.results[c]["out"]
        outs.append(r.reshape(2, 64, 4, LC).transpose(1, 2, 0, 3)
                    .reshape(B, VS))
    return np.concatenate(outs, axis=1)
